# revision 3
# baseline (speedup 1.0000x reference)
"""EntityEncoder forward kernel for 8 Trainium2 NeuronCores.

Reference computation (per entity row [unused, feat_a, azimuth, feat_b, type_id]):
    out[0]     = feat_a
    out[1]     = |az| / 180
    out[2]     = where(az >= -90, |90 - az|, 90 + |az + 180|) / 180
    out[3]     = feat_b
    out[4:128] = type_emb[int(type_id)]          # 124-wide embedding row

Sharding: data-parallel over the batch dim. Each of the 8 cores gets 32 of the
256 batches (65536 entities), the full embedding table, and produces its
65536x128 slice of the output. No collectives (forward only).

Per-core kernel layout ("p-major"): a tile covers 128*G entities; entity
e = tile*128*G + p*G + g lives on partition p, column-group g, so entity loads
and output stores are contiguous per partition.

The kernel is HBM/DMA-bus bound (gather reads + output stores dominate), so
everything on the wire is bf16: the table is rounded host-side to bf16 and
padded to 256-byte rows ([NTYPES, 128] bf16), `dma_gather` fetches one such
row per entity, the 4 head channels are computed in bf16 and merge-copied
over the row's pad slots, and the [NT, 128, G, 128] output is stored as bf16
(16 MB/core instead of 32). The host widens bf16->f32 exactly (bit shift) in
unshard. This halves both DMA directions vs the f32 version (65.75 ->
33.9 MB/core). Max elementwise error is ~2^-9, far inside the 2e-2 gate.

dma_gather contract: index-list position j writes output [j % 128, j // 128],
and the int16 index list lives wrapped in 16 partitions (position j at
[j % 16, j // 16]) replicated across all 128 partitions. The wrapped index
tensor is precomputed on the host during sharding (a pure relayout of entity
channel 4).

At the bf16 scale the per-call fixed costs matter: each dma_gather costs
~1us of SWDGE setup and each dma_start store ~0.6us of sequencer time, so
gathers are issued in 4 splits/tile (round-robined over 4 SWDGE queues =
8 GpSimd Q7 cores) and stores in 2 chunks/tile, vs 16/8 in the f32 version.
"""

import os
import sys

import numpy as np

for _p in ("/opt/trn_rl_repo", "/root/.axon_site/_ro/trn_rl_repo"):
    if os.path.isdir(_p) and _p not in sys.path:
        sys.path.append(_p)

B, N, EMB, NTYPES = 256, 2048, 128, 1000
NCORES = 8
BP = B // NCORES            # batches per core
E_CORE = BP * N             # entities per core (65536)
G = 64                      # entities per partition per tile
TILE_E = 128 * G            # entities per tile (8192)
NT = E_CORE // TILE_E       # tiles per core (8)
WCOLS = TILE_E // 16        # wrapped index columns per tile (512)

INV180 = float(np.float32(1.0) / np.float32(180.0))


def _f32_to_bf16(a):
    """Round-to-nearest-even f32 -> bf16, returned as uint16."""
    u = np.ascontiguousarray(a, dtype=np.float32).view(np.uint32)
    rounded = (u + 0x7FFF + ((u >> 16) & 1)) >> 16
    return rounded.astype(np.uint16)


def _bf16_to_f32(u16):
    """Exact bf16 (as uint16) -> f32 widening."""
    return (u16.astype(np.uint32) << 16).view(np.float32)


def build_nc(nt_run=NT, repeats=1, parts=("load", "gather", "compute", "store"),
             got_bufs=4, gather_queues=4, gather_splits=16, cs_splits=2,
             store_mdld=1024, idx_load_splits=2,
             store_alt=False, gather_sp=True, store_sp=False):
    import concourse.bacc as bacc
    import concourse.mybir as mybir
    import concourse.tile as tile

    bf16 = mybir.dt.bfloat16
    i16 = mybir.dt.int16
    Alu = mybir.AluOpType
    Act = mybir.ActivationFunctionType

    nc = bacc.Bacc(num_swdge_queues=gather_queues)
    table = nc.declare_dram_parameter("type_emb_pad", [NTYPES, EMB], bf16, isOutput=False)
    out = nc.declare_dram_parameter("out", [NT, 128, G, EMB], bf16, isOutput=True)
    ents_t = nc.declare_dram_parameter("ents_t", [128, NT * G, 3], bf16, isOutput=False)
    idxf = nc.declare_dram_parameter("idx_flat", [128, NT * WCOLS], i16, isOutput=False)

    with tile.TileContext(nc) as tc:
        with (
            tc.tile_pool(name="entp", bufs=2) as entp,
            tc.tile_pool(name="gotp", bufs=got_bufs) as gotp,
            tc.tile_pool(name="constp", bufs=1) as constp,
        ):
            neg_half = constp.tile([128, 1], bf16, tag="neg_half")
            nc.vector.memset(neg_half[:], -0.5)
            for rep in range(repeats):
                # Two big contiguous loads per repeat cover every tile's
                # entities (3 live channels) and wrapped gather indices.
                ent_all = entp.tile([128, NT * G, 3], bf16, tag="ent_all")
                idx_all = entp.tile([128, NT * WCOLS], i16, tag="idx_all")
                if "load" in parts:
                    nc.scalar.dma_start(out=ent_all[:], in_=ents_t[:])
                ic = NT * WCOLS // idx_load_splits
                for i in range(idx_load_splits):
                    nc.scalar.dma_start(out=idx_all[:, i * ic:(i + 1) * ic],
                                        in_=idxf[:, i * ic:(i + 1) * ic])
                if "compute" in parts:
                    # Head channels for ALL tiles in one pass; per-chunk
                    # work below is then a single 4-channel merge copy.
                    head_all = entp.tile([128, NT * G, 4], bf16, tag="head_all")
                    ta = entp.tile([128, NT * G], bf16, tag="ta")
                    tb = entp.tile([128, NT * G], bf16, tag="tb")
                    az = ent_all[:, :, 1]
                    nc.vector.tensor_copy(out=head_all[:, :, 0], in_=ent_all[:, :, 0])
                    nc.vector.tensor_copy(out=head_all[:, :, 3], in_=ent_all[:, :, 2])
                    # out1 = |az|/180 ; out2 = min(|az/180-0.5|, |az/180+1|+0.5)
                    nc.scalar.activation(head_all[:, :, 1], az, Act.Abs,
                                         bias=0.0, scale=INV180)
                    nc.scalar.activation(ta[:], az, Act.Abs, bias=neg_half[:],
                                         scale=INV180)
                    nc.scalar.activation(tb[:], az, Act.Abs, bias=1.0, scale=INV180)
                    nc.vector.scalar_tensor_tensor(
                        out=head_all[:, :, 2], in0=tb[:], scalar=0.5, in1=ta[:],
                        op0=Alu.add, op1=Alu.min,
                    )
                for t in range(nt_run):
                    got = gotp.tile([128, G, EMB], bf16, tag="got")
                    if "gather" not in parts:
                        nc.vector.memset(got[:, 0, 0:1], 0.0)  # mark written (ablations)
                    if "gather" in parts:
                        # Gather position j fills got[j%128, j//128]; a split at
                        # j0 = TILE_E*s/splits is a clean split of the idx list
                        # (cols j0//16:) and of got's column groups (g j0//128:).
                        ns = gather_splits
                        for s in range(ns):
                            iap = idx_all[:, t * WCOLS + s * (WCOLS // ns):
                                          t * WCOLS + (s + 1) * (WCOLS // ns)]
                            nc.gpsimd.dma_gather(
                                out_ap=got[:, s * (G // ns):(s + 1) * (G // ns), :],
                                in_ap=table[:, :],
                                idxs_ap=iap,
                                num_idxs=TILE_E // ns,
                                num_idxs_reg=TILE_E // ns,
                                elem_size=EMB,
                                single_packet=gather_sp,
                                queue_num=(t * ns + s) % gather_queues,
                            )

                    # Compute + store in column-group chunks: chunk c only waits
                    # on the gather splits covering its columns, so the
                    # tile-wide barrier disappears and stores stay contiguous.
                    gc = G // cs_splits
                    for c in range(cs_splits):
                        sl = slice(c * gc, (c + 1) * gc)
                        if "compute" in parts:
                            esl = slice(t * G + c * gc, t * G + (c + 1) * gc)
                            nc.vector.tensor_copy(out=got[:, sl, 0:4],
                                                  in_=head_all[:, esl, :])
                        if "store" in parts:
                            seng = nc.scalar if (store_alt and c % 2) else nc.sync
                            seng.dma_start(out=out[t][:, sl, :], in_=got[:, sl, :],
                                           max_dma_last_dim=store_mdld,
                                           single_packet=store_sp)
    nc.compile()
    return nc


def shard_inputs(entities, type_emb):
    """Full inputs -> per-core in_maps (p-major entity layout per tile)."""
    import ml_dtypes

    entities = np.ascontiguousarray(entities, dtype=np.float32)
    type_emb = np.ascontiguousarray(type_emb, dtype=np.float32)

    # bf16 table, rows zero-padded to 256B so a gathered row is exactly one
    # output row (4 pad slots the head-channel merge overwrites).
    table_pad = np.zeros((NTYPES, EMB), dtype=np.uint16)
    table_pad[:, 4:] = _f32_to_bf16(type_emb)
    table_pad = table_pad.view(ml_dtypes.bfloat16)

    ents = entities.reshape(NCORES, NT, 128, G, 5)

    # Wrapped gather indices: position j of tile t must hold the type id of
    # the entity at out[j%128, j//128] (= p-major entity p*G + g with
    # p = j%128, g = j//128), stored at [j%16, j//16], replicated x8.
    ids = ents[:, :, :, :, 4].astype(np.int16)          # [NCORES, NT, 128(p), G]
    pos = ids.transpose(0, 1, 3, 2)                     # v[j] with j = g*128 + p
    pos = pos.reshape(NCORES, NT, TILE_E)               # index value at position j
    wrapped = pos.reshape(NCORES, NT, WCOLS, 16).transpose(0, 1, 3, 2)  # [.., 16, WCOLS]
    wrapped = np.ascontiguousarray(np.tile(wrapped, (1, 1, 8, 1)))      # [.., 128, WCOLS]

    # Upfront-load layouts: all tiles' live entity channels / indices packed
    # partition-major so one contiguous DMA per repeat loads everything.
    ents_t = np.ascontiguousarray(
        _f32_to_bf16(ents[:, :, :, :, 1:4]).transpose(0, 2, 1, 3, 4)
        .reshape(NCORES, 128, NT * G, 3)
    ).view(ml_dtypes.bfloat16)
    idx_flat = np.ascontiguousarray(
        wrapped.transpose(0, 2, 1, 3).reshape(NCORES, 128, NT * WCOLS)
    )

    return [
        {"type_emb_pad": table_pad, "ents_t": ents_t[c], "idx_flat": idx_flat[c]}
        for c in range(NCORES)
    ]


def unshard_output(results):
    """Per-core result dicts -> full [B, N, EMB] f32 output (exact bf16 widen)."""
    outs = [
        _bf16_to_f32(np.asarray(r["out"]).view(np.uint16)).reshape(BP, N, EMB)
        for r in results
    ]
    return np.concatenate(outs, axis=0)


def _spot_check(out, entities, type_emb, n=256):
    """Cheap host-side sanity check on a random sample of entities: the
    embedding slots must match the bf16-rounded table row, slots 0/3 the
    bf16-rounded input channels."""
    rng = np.random.default_rng(1234)
    ef = entities.reshape(-1, 5)
    of = out.reshape(-1, EMB)
    sel = rng.integers(0, ef.shape[0], n)
    ids = ef[sel, 4].astype(np.int64)
    if not np.array_equal(of[sel, 4:], _bf16_to_f32(_f32_to_bf16(type_emb[ids]))):
        return False
    if not np.array_equal(of[sel, 0], _bf16_to_f32(_f32_to_bf16(ef[sel, 1]))):
        return False
    return bool(np.isfinite(of[sel]).all())


def kernel(entities, type_emb):
    from concourse.bass_utils import run_bass_kernel_spmd

    entities = np.ascontiguousarray(entities, dtype=np.float32)
    type_emb = np.ascontiguousarray(type_emb, dtype=np.float32)
    nc = build_nc()
    in_maps = shard_inputs(entities, type_emb)
    last_err = None
    for _attempt in range(3):
        try:
            res = run_bass_kernel_spmd(nc, in_maps, list(range(NCORES)))
            out = unshard_output(res.results)
            if _spot_check(out, entities, type_emb):
                return out
            last_err = RuntimeError("kernel output failed spot check")
        except Exception as e:  # flaky device sessions: retry
            last_err = e
    raise last_err


# revision 8
# speedup vs baseline: 1.4118x; 1.4118x over previous
"""EntityEncoder forward kernel for 8 Trainium2 NeuronCores.

Reference computation (per entity row [unused, feat_a, azimuth, feat_b, type_id]):
    out[0]     = feat_a
    out[1]     = |az| / 180
    out[2]     = where(az >= -90, |90 - az|, 90 + |az + 180|) / 180
    out[3]     = feat_b
    out[4:128] = type_emb[int(type_id)]          # 124-wide embedding row

Sharding: data-parallel over the batch dim. Each of the 8 cores gets 32 of the
256 batches (65536 entities), the full embedding table, and produces its
65536x128 slice of the output. No collectives (forward only).

Per-core kernel layout ("p-major"): a tile covers 128*G entities; entity
e = tile*128*G + p*G + g lives on partition p, column-group g, so entity loads
and output stores are contiguous per partition.

The kernel is HBM/DMA-bus bound (gather reads + output stores dominate), so
the bulk wire traffic is bf16: the table is rounded host-side to bf16 and
padded to 256-byte rows ([NTYPES, 128] bf16), `dma_gather` fetches one such
row per entity, the 4 head channels are merge-copied over the row's pad
slots, and the [NT, 128, G, 128] output is stored as bf16 (16 MB/core
instead of 32). The host widens bf16->f32 exactly (bit shift) in unshard.
This halves both DMA directions vs the f32 version (65.75 -> 34.3 MB/core).

Precision: entities stay f32 and all azimuth math runs in f32 (bf16 azimuth
input would turn the |az - 90| cancellation into ~1e-3 absolute error on
near-zero dist_east outputs); each output element then suffers exactly one
bf16 round-to-nearest (rel <= 2^-9), so both the global-scale and
elementwise relative error stay ~2e-3, inside the 2e-2 gate with margin.

dma_gather contract: index-list position j writes output [j % 128, j // 128],
and the int16 index list lives wrapped in 16 partitions (position j at
[j % 16, j // 16]) replicated across all 128 partitions (queue q's Q7 pair
reads its own 32-partition stripe). The wrapped index tensor is precomputed
on the host during sharding (a pure relayout of entity channel 4).

Performance structure (same-session A/Bs; absolute times swing 2-4x with
device phase, f32 baseline 277us -> this config 35-62us in its phases):
- bf16 wire traffic is the big lever (2.8x vs f32 in-session).
- Gathers go in 1024-index splits (2048 wedges the device regardless of
  ring carveout; 512 doubles the per-call ~1us SWDGE gen cost) spread over
  4 SWDGE queues = 8 GpSimd Q7 descriptor-gen cores, with the descriptor
  ring carveout grown to 64KB/partition (dynamic_dma_scratch_size=65536,
  +20% vs the default 16KB carveout at this split size).
- Stores alternate between the SP and Activation HWDGE engines
  (store_alt; 2x in-session) in 2 chunks/tile of 2KB descriptors.
"""

import os
import sys

import numpy as np

for _p in ("/opt/trn_rl_repo", "/root/.axon_site/_ro/trn_rl_repo"):
    if os.path.isdir(_p) and _p not in sys.path:
        sys.path.append(_p)

B, N, EMB, NTYPES = 256, 2048, 128, 1000
NCORES = 8
BP = B // NCORES            # batches per core
E_CORE = BP * N             # entities per core (65536)
G = 64                      # entities per partition per tile
TILE_E = 128 * G            # entities per tile (8192)
NT = E_CORE // TILE_E       # tiles per core (8)
WCOLS = TILE_E // 16        # wrapped index columns per tile (512)

INV180 = float(np.float32(1.0) / np.float32(180.0))


def _f32_to_bf16(a):
    """Round-to-nearest-even f32 -> bf16, returned as uint16."""
    u = np.ascontiguousarray(a, dtype=np.float32).view(np.uint32)
    rounded = (u + 0x7FFF + ((u >> 16) & 1)) >> 16
    return rounded.astype(np.uint16)


def _bf16_to_f32(u16):
    """Exact bf16 (as uint16) -> f32 widening."""
    return (u16.astype(np.uint32) << 16).view(np.float32)


def build_nc(nt_run=NT, repeats=1, parts=("load", "gather", "compute", "store"),
             got_bufs=4, gather_queues=4, gather_splits=8, cs_splits=2,
             store_mdld=1024, idx_load_splits=2, wire_dt="bf16",
             store_alt=True, store_eng3=False, gather_sp=True, store_sp=False,
             scratch=65536):
    import concourse.bacc as bacc
    import concourse.mybir as mybir
    import concourse.tile as tile

    f32 = mybir.dt.float32
    wdt = {"bf16": mybir.dt.bfloat16, "f32": f32}[wire_dt]
    i16 = mybir.dt.int16
    Alu = mybir.AluOpType
    Act = mybir.ActivationFunctionType

    kw = {"dynamic_dma_scratch_size": scratch} if scratch else {}
    nc = bacc.Bacc(num_swdge_queues=gather_queues, **kw)
    tname = "type_emb_pad" if wire_dt == "bf16" else "type_emb_pad32"
    table = nc.declare_dram_parameter(tname, [NTYPES, EMB], wdt, isOutput=False)
    out = nc.declare_dram_parameter("out", [NT, 128, G, EMB], wdt, isOutput=True)
    ents_t = nc.declare_dram_parameter("ents_t", [128, NT * G, 3], f32, isOutput=False)
    idxf = nc.declare_dram_parameter("idx_flat", [128, NT * WCOLS], i16, isOutput=False)

    with tile.TileContext(nc) as tc:
        with (
            tc.tile_pool(name="entp", bufs=2) as entp,
            tc.tile_pool(name="gotp", bufs=got_bufs) as gotp,
            tc.tile_pool(name="constp", bufs=1) as constp,
        ):
            neg_half = constp.tile([128, 1], f32, tag="neg_half")
            nc.vector.memset(neg_half[:], -0.5)
            for rep in range(repeats):
                # Two big contiguous loads per repeat cover every tile's
                # entities (3 live channels) and wrapped gather indices.
                ent_all = entp.tile([128, NT * G, 3], f32, tag="ent_all")
                idx_all = entp.tile([128, NT * WCOLS], i16, tag="idx_all")
                if "load" in parts:
                    nc.scalar.dma_start(out=ent_all[:], in_=ents_t[:])
                ic = NT * WCOLS // idx_load_splits
                for i in range(idx_load_splits):
                    nc.scalar.dma_start(out=idx_all[:, i * ic:(i + 1) * ic],
                                        in_=idxf[:, i * ic:(i + 1) * ic])
                if "compute" in parts:
                    # Head channels for ALL tiles in one pass (f32 math, one
                    # bf16 rounding on write); per-chunk work below is then a
                    # single 4-channel merge copy.
                    head_all = entp.tile([128, NT * G, 4], wdt, tag="head_all")
                    ta = entp.tile([128, NT * G], f32, tag="ta")
                    tb = entp.tile([128, NT * G], f32, tag="tb")
                    az = ent_all[:, :, 1]
                    nc.vector.tensor_copy(out=head_all[:, :, 0], in_=ent_all[:, :, 0])
                    nc.vector.tensor_copy(out=head_all[:, :, 3], in_=ent_all[:, :, 2])
                    # out1 = |az|/180 ; out2 = min(|az/180-0.5|, |az/180+1|+0.5)
                    nc.scalar.activation(head_all[:, :, 1], az, Act.Abs,
                                         bias=0.0, scale=INV180)
                    nc.scalar.activation(ta[:], az, Act.Abs, bias=neg_half[:],
                                         scale=INV180)
                    nc.scalar.activation(tb[:], az, Act.Abs, bias=1.0, scale=INV180)
                    nc.vector.scalar_tensor_tensor(
                        out=head_all[:, :, 2], in0=tb[:], scalar=0.5, in1=ta[:],
                        op0=Alu.add, op1=Alu.min,
                    )
                for t in range(nt_run):
                    got = gotp.tile([128, G, EMB], wdt, tag="got")
                    if "gather" not in parts:
                        nc.vector.memset(got[:, 0, 0:1], 0.0)  # mark written (ablations)
                    if "gather" in parts:
                        # Gather position j fills got[j%128, j//128]; a split at
                        # j0 = TILE_E*s/splits is a clean split of the idx list
                        # (cols j0//16:) and of got's column groups (g j0//128:).
                        ns = gather_splits
                        for s in range(ns):
                            iap = idx_all[:, t * WCOLS + s * (WCOLS // ns):
                                          t * WCOLS + (s + 1) * (WCOLS // ns)]
                            nc.gpsimd.dma_gather(
                                out_ap=got[:, s * (G // ns):(s + 1) * (G // ns), :],
                                in_ap=table[:, :],
                                idxs_ap=iap,
                                num_idxs=TILE_E // ns,
                                num_idxs_reg=TILE_E // ns,
                                elem_size=EMB,
                                single_packet=gather_sp,
                                queue_num=(t * ns + s) % gather_queues,
                            )

                    # Compute + store in column-group chunks: chunk c only waits
                    # on the gather splits covering its columns, so the
                    # tile-wide barrier disappears and stores stay contiguous.
                    gc = G // cs_splits
                    for c in range(cs_splits):
                        sl = slice(c * gc, (c + 1) * gc)
                        if "compute" in parts:
                            esl = slice(t * G + c * gc, t * G + (c + 1) * gc)
                            nc.vector.tensor_copy(out=got[:, sl, 0:4],
                                                  in_=head_all[:, esl, :])
                        if "store" in parts:
                            si = (t * cs_splits + c)
                            if store_eng3:
                                seng = (nc.sync, nc.scalar, nc.vector)[si % 3]
                            elif store_alt:
                                seng = (nc.sync, nc.scalar)[si % 2]
                            else:
                                seng = nc.sync
                            seng.dma_start(out=out[t][:, sl, :], in_=got[:, sl, :],
                                           max_dma_last_dim=store_mdld,
                                           single_packet=store_sp)
    nc.compile()
    return nc


def shard_inputs(entities, type_emb):
    """Full inputs -> per-core in_maps (p-major entity layout per tile).

    The maps carry both the bf16 and f32 padded tables so either wire_dt
    build can run from the same maps (each NEFF only reads its own keys).
    """
    import ml_dtypes

    entities = np.ascontiguousarray(entities, dtype=np.float32)
    type_emb = np.ascontiguousarray(type_emb, dtype=np.float32)

    # Table rows zero-padded to one gather element = one output row (4 pad
    # slots the head-channel merge overwrites). bf16 rows are 256B.
    table_pad = np.zeros((NTYPES, EMB), dtype=np.uint16)
    table_pad[:, 4:] = _f32_to_bf16(type_emb)
    table_pad = table_pad.view(ml_dtypes.bfloat16)
    table_pad32 = np.zeros((NTYPES, EMB), dtype=np.float32)
    table_pad32[:, 4:] = type_emb

    ents = entities.reshape(NCORES, NT, 128, G, 5)

    # Wrapped gather indices: position j of tile t must hold the type id of
    # the entity at out[j%128, j//128] (= p-major entity p*G + g with
    # p = j%128, g = j//128), stored at [j%16, j//16], replicated x8.
    ids = ents[:, :, :, :, 4].astype(np.int16)          # [NCORES, NT, 128(p), G]
    pos = ids.transpose(0, 1, 3, 2)                     # v[j] with j = g*128 + p
    pos = pos.reshape(NCORES, NT, TILE_E)               # index value at position j
    wrapped = pos.reshape(NCORES, NT, WCOLS, 16).transpose(0, 1, 3, 2)  # [.., 16, WCOLS]
    wrapped = np.ascontiguousarray(np.tile(wrapped, (1, 1, 8, 1)))      # [.., 128, WCOLS]

    # Upfront-load layouts: all tiles' live entity channels / indices packed
    # partition-major so one contiguous DMA per repeat loads everything.
    ents_t = np.ascontiguousarray(
        ents[:, :, :, :, 1:4].transpose(0, 2, 1, 3, 4).reshape(NCORES, 128, NT * G, 3)
    )
    idx_flat = np.ascontiguousarray(
        wrapped.transpose(0, 2, 1, 3).reshape(NCORES, 128, NT * WCOLS)
    )

    return [
        {"type_emb_pad": table_pad, "type_emb_pad32": table_pad32,
         "ents_t": ents_t[c], "idx_flat": idx_flat[c]}
        for c in range(NCORES)
    ]


def unshard_output(results):
    """Per-core result dicts -> full [B, N, EMB] f32 output (exact bf16 widen)."""
    outs = []
    for r in results:
        o = np.asarray(r["out"])
        if o.dtype != np.float32:
            o = _bf16_to_f32(o.view(np.uint16))
        outs.append(o.reshape(BP, N, EMB))
    return np.concatenate(outs, axis=0)


def _spot_check(out, entities, type_emb, n=256):
    """Cheap host-side sanity check on a random sample of entities: the
    embedding slots must match the bf16-rounded table row, slots 0/3 the
    bf16-rounded input channels."""
    rng = np.random.default_rng(1234)
    ef = entities.reshape(-1, 5)
    of = out.reshape(-1, EMB)
    sel = rng.integers(0, ef.shape[0], n)
    ids = ef[sel, 4].astype(np.int64)
    if not np.array_equal(of[sel, 4:], _bf16_to_f32(_f32_to_bf16(type_emb[ids]))):
        return False
    if not np.array_equal(of[sel, 0], _bf16_to_f32(_f32_to_bf16(ef[sel, 1]))):
        return False
    return bool(np.isfinite(of[sel]).all())


def kernel(entities, type_emb):
    from concourse.bass_utils import run_bass_kernel_spmd

    entities = np.ascontiguousarray(entities, dtype=np.float32)
    type_emb = np.ascontiguousarray(type_emb, dtype=np.float32)
    nc = build_nc()
    in_maps = shard_inputs(entities, type_emb)
    last_err = None
    for _attempt in range(3):
        try:
            res = run_bass_kernel_spmd(nc, in_maps, list(range(NCORES)))
            out = unshard_output(res.results)
            if _spot_check(out, entities, type_emb):
                return out
            last_err = RuntimeError("kernel output failed spot check")
        except Exception as e:  # flaky device sessions: retry
            last_err = e
    raise last_err


# revision 12
# speedup vs baseline: 1.5769x; 1.1170x over previous
"""EntityEncoder forward kernel for 8 Trainium2 NeuronCores.

Reference computation (per entity row [unused, feat_a, azimuth, feat_b, type_id]):
    out[0]     = feat_a
    out[1]     = |az| / 180
    out[2]     = where(az >= -90, |90 - az|, 90 + |az + 180|) / 180
    out[3]     = feat_b
    out[4:128] = type_emb[int(type_id)]          # 124-wide embedding row

Sharding: data-parallel over the batch dim. Each of the 8 cores gets 32 of the
256 batches (65536 entities), the full embedding table, and produces its
65536x128 slice of the output. No collectives (forward only).

Per-core kernel layout ("p-major"): a tile covers 128*G entities; entity
e = tile*128*G + p*G + g lives on partition p, column-group g, so entity loads
and output stores are contiguous per partition.

The kernel is HBM/DMA-bus bound (gather reads + output stores dominate), so
the bulk wire traffic is bf16: the table is rounded host-side to bf16 and
padded to 256-byte rows ([NTYPES, 128] bf16), `dma_gather` fetches one such
row per entity, the 4 head channels are merge-copied over the row's pad
slots, and the [NT, 128, G, 128] output is stored as bf16 (16 MB/core
instead of 32). The host widens bf16->f32 exactly (bit shift) in unshard.
This halves both DMA directions vs the f32 version (65.75 -> 34.3 MB/core).

Precision: entities stay f32 and all azimuth math runs in f32 (bf16 azimuth
input would turn the |az - 90| cancellation into ~1e-3 absolute error on
near-zero dist_east outputs); each output element then suffers exactly one
bf16 round-to-nearest (rel <= 2^-9), so both the global-scale and
elementwise relative error stay ~2e-3, inside the 2e-2 gate with margin.

dma_gather contract: index-list position j writes output [j % 128, j // 128],
and the int16 index list lives wrapped in 16 partitions (position j at
[j % 16, j // 16]) replicated across all 128 partitions (queue q's Q7 pair
reads its own 32-partition stripe). The wrapped index tensor is precomputed
on the host during sharding (a pure relayout of entity channel 4).

Performance structure (same-session A/Bs; absolute times swing 2-4x with
device phase, f32 baseline 277us -> this config 35-62us in its phases):
- bf16 wire traffic is the big lever (2.8x vs f32 in-session).
- Gathers go in 1024-index splits (2048 wedges the device regardless of
  ring carveout; 512 doubles the per-call ~1us SWDGE gen cost) spread over
  4 SWDGE queues = 8 GpSimd Q7 descriptor-gen cores, with the descriptor
  ring carveout grown to 64KB/partition (dynamic_dma_scratch_size=65536,
  +20% vs the default 16KB carveout at this split size).
- Stores alternate between the SP and Activation HWDGE engines
  (store_alt; 2x in-session) in 2 chunks/tile of 2KB descriptors.
"""

import os
import sys

import numpy as np

for _p in ("/opt/trn_rl_repo", "/root/.axon_site/_ro/trn_rl_repo"):
    if os.path.isdir(_p) and _p not in sys.path:
        sys.path.append(_p)

B, N, EMB, NTYPES = 256, 2048, 128, 1000
NCORES = 8
BP = B // NCORES            # batches per core
E_CORE = BP * N             # entities per core (65536)
G = 64                      # entities per partition per tile
TILE_E = 128 * G            # entities per tile (8192)
NT = E_CORE // TILE_E       # tiles per core (8)
WCOLS = TILE_E // 16        # wrapped index columns per tile (512)

INV180 = float(np.float32(1.0) / np.float32(180.0))


def _f32_to_bf16(a):
    """Round-to-nearest-even f32 -> bf16, returned as uint16."""
    u = np.ascontiguousarray(a, dtype=np.float32).view(np.uint32)
    rounded = (u + 0x7FFF + ((u >> 16) & 1)) >> 16
    return rounded.astype(np.uint16)


def _bf16_to_f32(u16):
    """Exact bf16 (as uint16) -> f32 widening."""
    return (u16.astype(np.uint32) << 16).view(np.float32)


def build_nc(nt_run=NT, repeats=1, parts=("load", "gather", "compute", "store"),
             got_bufs=4, gather_queues=4, gather_splits=8, cs_splits=2,
             store_mdld=1024, idx_load_splits=2, wire_dt="bf16",
             store_alt=True, store_eng3=False, gather_sp=True, store_sp=False,
             scratch=65536, table_reps=1):
    import concourse.bacc as bacc
    import concourse.mybir as mybir
    import concourse.tile as tile

    f32 = mybir.dt.float32
    wdt = {"bf16": mybir.dt.bfloat16, "f32": f32}[wire_dt]
    i16 = mybir.dt.int16
    Alu = mybir.AluOpType
    Act = mybir.ActivationFunctionType

    kw = {"dynamic_dma_scratch_size": scratch} if scratch else {}
    nc = bacc.Bacc(num_swdge_queues=gather_queues, **kw)
    tname = "type_emb_pad" if wire_dt == "bf16" else "type_emb_pad32"
    if table_reps > 1:
        # K copies of the table; gather splits round-robin over them to
        # spread random-row HBM pressure across more DRAM rows/banks.
        tableR = nc.declare_dram_parameter("type_emb_padR",
                                           [table_reps, NTYPES, EMB], wdt,
                                           isOutput=False)
        table = None
    else:
        table = nc.declare_dram_parameter(tname, [NTYPES, EMB], wdt, isOutput=False)
    out = nc.declare_dram_parameter("out", [NT, 128, G, EMB], wdt, isOutput=True)
    ents_t = nc.declare_dram_parameter("ents_t", [128, NT * G, 3], f32, isOutput=False)
    idxf = nc.declare_dram_parameter("idx_flat", [128, NT * WCOLS], i16, isOutput=False)

    with tile.TileContext(nc) as tc:
        with (
            tc.tile_pool(name="entp", bufs=2) as entp,
            tc.tile_pool(name="gotp", bufs=got_bufs) as gotp,
            tc.tile_pool(name="constp", bufs=1) as constp,
        ):
            neg_half = constp.tile([128, 1], f32, tag="neg_half")
            nc.vector.memset(neg_half[:], -0.5)
            for rep in range(repeats):
                # Two big contiguous loads per repeat cover every tile's
                # entities (3 live channels) and wrapped gather indices.
                ent_all = entp.tile([128, NT * G, 3], f32, tag="ent_all")
                idx_all = entp.tile([128, NT * WCOLS], i16, tag="idx_all")
                if "load" in parts:
                    nc.scalar.dma_start(out=ent_all[:], in_=ents_t[:])
                ic = NT * WCOLS // idx_load_splits
                for i in range(idx_load_splits):
                    nc.scalar.dma_start(out=idx_all[:, i * ic:(i + 1) * ic],
                                        in_=idxf[:, i * ic:(i + 1) * ic])
                if "compute" in parts:
                    # Head channels for ALL tiles in one pass (f32 math, one
                    # bf16 rounding on write); per-chunk work below is then a
                    # single 4-channel merge copy.
                    head_all = entp.tile([128, NT * G, 4], wdt, tag="head_all")
                    ta = entp.tile([128, NT * G], f32, tag="ta")
                    tb = entp.tile([128, NT * G], f32, tag="tb")
                    az = ent_all[:, :, 1]
                    nc.vector.tensor_copy(out=head_all[:, :, 0], in_=ent_all[:, :, 0])
                    nc.vector.tensor_copy(out=head_all[:, :, 3], in_=ent_all[:, :, 2])
                    # out1 = |az|/180 ; out2 = min(|az/180-0.5|, |az/180+1|+0.5)
                    nc.scalar.activation(head_all[:, :, 1], az, Act.Abs,
                                         bias=0.0, scale=INV180)
                    nc.scalar.activation(ta[:], az, Act.Abs, bias=neg_half[:],
                                         scale=INV180)
                    nc.scalar.activation(tb[:], az, Act.Abs, bias=1.0, scale=INV180)
                    nc.vector.scalar_tensor_tensor(
                        out=head_all[:, :, 2], in0=tb[:], scalar=0.5, in1=ta[:],
                        op0=Alu.add, op1=Alu.min,
                    )
                for t in range(nt_run):
                    got = gotp.tile([128, G, EMB], wdt, tag="got")
                    if "gather" not in parts:
                        nc.vector.memset(got[:, 0, 0:1], 0.0)  # mark written (ablations)
                    if "gather" in parts:
                        # Gather position j fills got[j%128, j//128]; a split at
                        # j0 = TILE_E*s/splits is a clean split of the idx list
                        # (cols j0//16:) and of got's column groups (g j0//128:).
                        ns = gather_splits
                        for s in range(ns):
                            iap = idx_all[:, t * WCOLS + s * (WCOLS // ns):
                                          t * WCOLS + (s + 1) * (WCOLS // ns)]
                            tap = (table[:, :] if table_reps == 1
                                   else tableR[(t * ns + s) % table_reps])
                            nc.gpsimd.dma_gather(
                                out_ap=got[:, s * (G // ns):(s + 1) * (G // ns), :],
                                in_ap=tap,
                                idxs_ap=iap,
                                num_idxs=TILE_E // ns,
                                num_idxs_reg=TILE_E // ns,
                                elem_size=EMB,
                                single_packet=gather_sp,
                                queue_num=(t * ns + s) % gather_queues,
                            )

                    # Compute + store in column-group chunks: chunk c only waits
                    # on the gather splits covering its columns, so the
                    # tile-wide barrier disappears and stores stay contiguous.
                    gc = G // cs_splits
                    for c in range(cs_splits):
                        sl = slice(c * gc, (c + 1) * gc)
                        if "compute" in parts:
                            esl = slice(t * G + c * gc, t * G + (c + 1) * gc)
                            nc.vector.tensor_copy(out=got[:, sl, 0:4],
                                                  in_=head_all[:, esl, :])
                        if "store" in parts:
                            si = (t * cs_splits + c)
                            if store_eng3:
                                seng = (nc.sync, nc.scalar, nc.vector)[si % 3]
                            elif store_alt:
                                seng = (nc.sync, nc.scalar)[si % 2]
                            else:
                                seng = nc.sync
                            seng.dma_start(out=out[t][:, sl, :], in_=got[:, sl, :],
                                           max_dma_last_dim=store_mdld,
                                           single_packet=store_sp)
    nc.compile()
    return nc


def shard_inputs(entities, type_emb):
    """Full inputs -> per-core in_maps (p-major entity layout per tile).

    The maps carry both the bf16 and f32 padded tables so either wire_dt
    build can run from the same maps (each NEFF only reads its own keys).
    """
    import ml_dtypes

    entities = np.ascontiguousarray(entities, dtype=np.float32)
    type_emb = np.ascontiguousarray(type_emb, dtype=np.float32)

    # Table rows zero-padded to one gather element = one output row (4 pad
    # slots the head-channel merge overwrites). bf16 rows are 256B.
    table_pad = np.zeros((NTYPES, EMB), dtype=np.uint16)
    table_pad[:, 4:] = _f32_to_bf16(type_emb)
    table_pad = table_pad.view(ml_dtypes.bfloat16)
    table_pad32 = np.zeros((NTYPES, EMB), dtype=np.float32)
    table_pad32[:, 4:] = type_emb

    ents = entities.reshape(NCORES, NT, 128, G, 5)

    # Wrapped gather indices: position j of tile t must hold the type id of
    # the entity at out[j%128, j//128] (= p-major entity p*G + g with
    # p = j%128, g = j//128), stored at [j%16, j//16], replicated x8.
    ids = ents[:, :, :, :, 4].astype(np.int16)          # [NCORES, NT, 128(p), G]
    pos = ids.transpose(0, 1, 3, 2)                     # v[j] with j = g*128 + p
    pos = pos.reshape(NCORES, NT, TILE_E)               # index value at position j
    wrapped = pos.reshape(NCORES, NT, WCOLS, 16).transpose(0, 1, 3, 2)  # [.., 16, WCOLS]
    wrapped = np.ascontiguousarray(np.tile(wrapped, (1, 1, 8, 1)))      # [.., 128, WCOLS]

    # Upfront-load layouts: all tiles' live entity channels / indices packed
    # partition-major so one contiguous DMA per repeat loads everything.
    ents_t = np.ascontiguousarray(
        ents[:, :, :, :, 1:4].transpose(0, 2, 1, 3, 4).reshape(NCORES, 128, NT * G, 3)
    )
    idx_flat = np.ascontiguousarray(
        wrapped.transpose(0, 2, 1, 3).reshape(NCORES, 128, NT * WCOLS)
    )

    table_padR = np.ascontiguousarray(np.tile(table_pad[None], (4, 1, 1)))

    return [
        {"type_emb_pad": table_pad, "type_emb_pad32": table_pad32,
         "type_emb_padR": table_padR,
         "ents_t": ents_t[c], "idx_flat": idx_flat[c]}
        for c in range(NCORES)
    ]


def unshard_output(results):
    """Per-core result dicts -> full [B, N, EMB] f32 output (exact bf16 widen)."""
    outs = []
    for r in results:
        o = np.asarray(r["out"])
        if o.dtype != np.float32:
            o = _bf16_to_f32(o.view(np.uint16))
        outs.append(o.reshape(BP, N, EMB))
    return np.concatenate(outs, axis=0)


def _spot_check(out, entities, type_emb, n=256):
    """Cheap host-side sanity check on a random sample of entities: the
    embedding slots must match the bf16-rounded table row, slots 0/3 the
    bf16-rounded input channels."""
    rng = np.random.default_rng(1234)
    ef = entities.reshape(-1, 5)
    of = out.reshape(-1, EMB)
    sel = rng.integers(0, ef.shape[0], n)
    ids = ef[sel, 4].astype(np.int64)
    if not np.array_equal(of[sel, 4:], _bf16_to_f32(_f32_to_bf16(type_emb[ids]))):
        return False
    if not np.array_equal(of[sel, 0], _bf16_to_f32(_f32_to_bf16(ef[sel, 1]))):
        return False
    return bool(np.isfinite(of[sel]).all())


def kernel(entities, type_emb):
    from concourse.bass_utils import run_bass_kernel_spmd

    entities = np.ascontiguousarray(entities, dtype=np.float32)
    type_emb = np.ascontiguousarray(type_emb, dtype=np.float32)
    nc = build_nc()
    in_maps = shard_inputs(entities, type_emb)
    last_err = None
    for _attempt in range(3):
        try:
            res = run_bass_kernel_spmd(nc, in_maps, list(range(NCORES)))
            out = unshard_output(res.results)
            if _spot_check(out, entities, type_emb):
                return out
            last_err = RuntimeError("kernel output failed spot check")
        except Exception as e:  # flaky device sessions: retry
            last_err = e
    raise last_err


# revision 13
# speedup vs baseline: 1.6570x; 1.0508x over previous
"""EntityEncoder forward kernel for 8 Trainium2 NeuronCores.

Reference computation (per entity row [unused, feat_a, azimuth, feat_b, type_id]):
    out[0]     = feat_a
    out[1]     = |az| / 180
    out[2]     = where(az >= -90, |90 - az|, 90 + |az + 180|) / 180
    out[3]     = feat_b
    out[4:128] = type_emb[int(type_id)]          # 124-wide embedding row

Sharding: data-parallel over the batch dim. Each of the 8 cores gets 32 of the
256 batches (65536 entities), the full embedding table, and produces its
65536x128 slice of the output. No collectives (forward only).

Per-core kernel layout ("p-major"): a tile covers 128*G entities; entity
e = tile*128*G + p*G + g lives on partition p, column-group g, so entity loads
and output stores are contiguous per partition.

The kernel is HBM/DMA-bus bound (gather reads + output stores dominate), so
the bulk wire traffic is bf16: the table is rounded host-side to bf16 and
padded to 256-byte rows ([NTYPES, 128] bf16), `dma_gather` fetches one such
row per entity, the 4 head channels are merge-copied over the row's pad
slots, and the [NT, 128, G, 128] output is stored as bf16 (16 MB/core
instead of 32). The host widens bf16->f32 exactly (bit shift) in unshard.
This halves both DMA directions vs the f32 version (65.75 -> 34.3 MB/core).

Precision: entities stay f32 and all azimuth math runs in f32 (bf16 azimuth
input would turn the |az - 90| cancellation into ~1e-3 absolute error on
near-zero dist_east outputs); each output element then suffers exactly one
bf16 round-to-nearest (rel <= 2^-9), so both the global-scale and
elementwise relative error stay ~2e-3, inside the 2e-2 gate with margin.

dma_gather contract: index-list position j writes output [j % 128, j // 128],
and the int16 index list lives wrapped in 16 partitions (position j at
[j % 16, j // 16]) replicated across all 128 partitions (queue q's Q7 pair
reads its own 32-partition stripe). The wrapped index tensor is precomputed
on the host during sharding (a pure relayout of entity channel 4).

Performance structure (same-session A/Bs; absolute times swing 2-4x with
device phase, f32 baseline 277us -> this config 35-62us in its phases):
- bf16 wire traffic is the big lever (2.8x vs f32 in-session).
- Gathers go in 1024-index splits (2048 wedges the device regardless of
  ring carveout; 512 doubles the per-call ~1us SWDGE gen cost) spread over
  4 SWDGE queues = 8 GpSimd Q7 descriptor-gen cores, with the descriptor
  ring carveout grown to 64KB/partition (dynamic_dma_scratch_size=65536,
  +20% vs the default 16KB carveout at this split size).
- Stores alternate between the SP and Activation HWDGE engines
  (store_alt; 2x in-session) in 2 chunks/tile of 2KB descriptors.
"""

import os
import sys

import numpy as np

for _p in ("/opt/trn_rl_repo", "/root/.axon_site/_ro/trn_rl_repo"):
    if os.path.isdir(_p) and _p not in sys.path:
        sys.path.append(_p)

B, N, EMB, NTYPES = 256, 2048, 128, 1000
NCORES = 8
BP = B // NCORES            # batches per core
E_CORE = BP * N             # entities per core (65536)
G = 64                      # entities per partition per tile
TILE_E = 128 * G            # entities per tile (8192)
NT = E_CORE // TILE_E       # tiles per core (8)
WCOLS = TILE_E // 16        # wrapped index columns per tile (512)

INV180 = float(np.float32(1.0) / np.float32(180.0))


def _f32_to_bf16(a):
    """Round-to-nearest-even f32 -> bf16, returned as uint16."""
    u = np.ascontiguousarray(a, dtype=np.float32).view(np.uint32)
    rounded = (u + 0x7FFF + ((u >> 16) & 1)) >> 16
    return rounded.astype(np.uint16)


def _bf16_to_f32(u16):
    """Exact bf16 (as uint16) -> f32 widening."""
    return (u16.astype(np.uint32) << 16).view(np.float32)


def build_nc(nt_run=NT, repeats=1, parts=("load", "gather", "compute", "store"),
             got_bufs=5, gather_queues=4, gather_splits=8, cs_splits=2,
             store_mdld=1024, idx_load_splits=2, wire_dt="bf16",
             store_alt=True, store_eng3=False, gather_sp=True, store_sp=False,
             scratch=65536, table_reps=1):
    import concourse.bacc as bacc
    import concourse.mybir as mybir
    import concourse.tile as tile

    f32 = mybir.dt.float32
    wdt = {"bf16": mybir.dt.bfloat16, "f32": f32}[wire_dt]
    i16 = mybir.dt.int16
    Alu = mybir.AluOpType
    Act = mybir.ActivationFunctionType

    kw = {"dynamic_dma_scratch_size": scratch} if scratch else {}
    nc = bacc.Bacc(num_swdge_queues=gather_queues, **kw)
    tname = "type_emb_pad" if wire_dt == "bf16" else "type_emb_pad32"
    if table_reps > 1:
        # K copies of the table; gather splits round-robin over them to
        # spread random-row HBM pressure across more DRAM rows/banks.
        tableR = nc.declare_dram_parameter("type_emb_padR",
                                           [table_reps, NTYPES, EMB], wdt,
                                           isOutput=False)
        table = None
    else:
        table = nc.declare_dram_parameter(tname, [NTYPES, EMB], wdt, isOutput=False)
    out = nc.declare_dram_parameter("out", [NT, 128, G, EMB], wdt, isOutput=True)
    ents_t = nc.declare_dram_parameter("ents_t", [128, NT * G, 3], f32, isOutput=False)
    idxf = nc.declare_dram_parameter("idx_flat", [128, NT * WCOLS], i16, isOutput=False)

    with tile.TileContext(nc) as tc:
        with (
            tc.tile_pool(name="entp", bufs=2) as entp,
            tc.tile_pool(name="gotp", bufs=got_bufs) as gotp,
            tc.tile_pool(name="constp", bufs=1) as constp,
        ):
            neg_half = constp.tile([128, 1], f32, tag="neg_half")
            nc.vector.memset(neg_half[:], -0.5)
            for rep in range(repeats):
                # Two big contiguous loads per repeat cover every tile's
                # entities (3 live channels) and wrapped gather indices.
                ent_all = entp.tile([128, NT * G, 3], f32, tag="ent_all")
                idx_all = entp.tile([128, NT * WCOLS], i16, tag="idx_all")
                if "load" in parts:
                    nc.scalar.dma_start(out=ent_all[:], in_=ents_t[:])
                ic = NT * WCOLS // idx_load_splits
                for i in range(idx_load_splits):
                    nc.scalar.dma_start(out=idx_all[:, i * ic:(i + 1) * ic],
                                        in_=idxf[:, i * ic:(i + 1) * ic])
                if "compute" in parts:
                    # Head channels for ALL tiles in one pass (f32 math, one
                    # bf16 rounding on write); per-chunk work below is then a
                    # single 4-channel merge copy.
                    head_all = entp.tile([128, NT * G, 4], wdt, tag="head_all")
                    ta = entp.tile([128, NT * G], f32, tag="ta")
                    tb = entp.tile([128, NT * G], f32, tag="tb")
                    az = ent_all[:, :, 1]
                    nc.vector.tensor_copy(out=head_all[:, :, 0], in_=ent_all[:, :, 0])
                    nc.vector.tensor_copy(out=head_all[:, :, 3], in_=ent_all[:, :, 2])
                    # out1 = |az|/180 ; out2 = min(|az/180-0.5|, |az/180+1|+0.5)
                    nc.scalar.activation(head_all[:, :, 1], az, Act.Abs,
                                         bias=0.0, scale=INV180)
                    nc.scalar.activation(ta[:], az, Act.Abs, bias=neg_half[:],
                                         scale=INV180)
                    nc.scalar.activation(tb[:], az, Act.Abs, bias=1.0, scale=INV180)
                    nc.vector.scalar_tensor_tensor(
                        out=head_all[:, :, 2], in0=tb[:], scalar=0.5, in1=ta[:],
                        op0=Alu.add, op1=Alu.min,
                    )
                for t in range(nt_run):
                    got = gotp.tile([128, G, EMB], wdt, tag="got")
                    if "gather" not in parts:
                        nc.vector.memset(got[:, 0, 0:1], 0.0)  # mark written (ablations)
                    if "gather" in parts:
                        # Gather position j fills got[j%128, j//128]; a split at
                        # j0 = TILE_E*s/splits is a clean split of the idx list
                        # (cols j0//16:) and of got's column groups (g j0//128:).
                        ns = gather_splits
                        for s in range(ns):
                            iap = idx_all[:, t * WCOLS + s * (WCOLS // ns):
                                          t * WCOLS + (s + 1) * (WCOLS // ns)]
                            tap = (table[:, :] if table_reps == 1
                                   else tableR[(t * ns + s) % table_reps])
                            nc.gpsimd.dma_gather(
                                out_ap=got[:, s * (G // ns):(s + 1) * (G // ns), :],
                                in_ap=tap,
                                idxs_ap=iap,
                                num_idxs=TILE_E // ns,
                                num_idxs_reg=TILE_E // ns,
                                elem_size=EMB,
                                single_packet=gather_sp,
                                queue_num=(t * ns + s) % gather_queues,
                            )

                    # Compute + store in column-group chunks: chunk c only waits
                    # on the gather splits covering its columns, so the
                    # tile-wide barrier disappears and stores stay contiguous.
                    gc = G // cs_splits
                    for c in range(cs_splits):
                        sl = slice(c * gc, (c + 1) * gc)
                        if "compute" in parts:
                            esl = slice(t * G + c * gc, t * G + (c + 1) * gc)
                            nc.vector.tensor_copy(out=got[:, sl, 0:4],
                                                  in_=head_all[:, esl, :])
                        if "store" in parts:
                            si = (t * cs_splits + c)
                            if store_eng3:
                                seng = (nc.sync, nc.scalar, nc.vector)[si % 3]
                            elif store_alt:
                                seng = (nc.sync, nc.scalar)[si % 2]
                            else:
                                seng = nc.sync
                            seng.dma_start(out=out[t][:, sl, :], in_=got[:, sl, :],
                                           max_dma_last_dim=store_mdld,
                                           single_packet=store_sp)
    nc.compile()
    return nc


def shard_inputs(entities, type_emb):
    """Full inputs -> per-core in_maps (p-major entity layout per tile).

    The maps carry both the bf16 and f32 padded tables so either wire_dt
    build can run from the same maps (each NEFF only reads its own keys).
    """
    import ml_dtypes

    entities = np.ascontiguousarray(entities, dtype=np.float32)
    type_emb = np.ascontiguousarray(type_emb, dtype=np.float32)

    # Table rows zero-padded to one gather element = one output row (4 pad
    # slots the head-channel merge overwrites). bf16 rows are 256B.
    table_pad = np.zeros((NTYPES, EMB), dtype=np.uint16)
    table_pad[:, 4:] = _f32_to_bf16(type_emb)
    table_pad = table_pad.view(ml_dtypes.bfloat16)
    table_pad32 = np.zeros((NTYPES, EMB), dtype=np.float32)
    table_pad32[:, 4:] = type_emb

    ents = entities.reshape(NCORES, NT, 128, G, 5)

    # Wrapped gather indices: position j of tile t must hold the type id of
    # the entity at out[j%128, j//128] (= p-major entity p*G + g with
    # p = j%128, g = j//128), stored at [j%16, j//16], replicated x8.
    ids = ents[:, :, :, :, 4].astype(np.int16)          # [NCORES, NT, 128(p), G]
    pos = ids.transpose(0, 1, 3, 2)                     # v[j] with j = g*128 + p
    pos = pos.reshape(NCORES, NT, TILE_E)               # index value at position j
    wrapped = pos.reshape(NCORES, NT, WCOLS, 16).transpose(0, 1, 3, 2)  # [.., 16, WCOLS]
    wrapped = np.ascontiguousarray(np.tile(wrapped, (1, 1, 8, 1)))      # [.., 128, WCOLS]

    # Upfront-load layouts: all tiles' live entity channels / indices packed
    # partition-major so one contiguous DMA per repeat loads everything.
    ents_t = np.ascontiguousarray(
        ents[:, :, :, :, 1:4].transpose(0, 2, 1, 3, 4).reshape(NCORES, 128, NT * G, 3)
    )
    idx_flat = np.ascontiguousarray(
        wrapped.transpose(0, 2, 1, 3).reshape(NCORES, 128, NT * WCOLS)
    )

    table_padR = np.ascontiguousarray(np.tile(table_pad[None], (4, 1, 1)))

    return [
        {"type_emb_pad": table_pad, "type_emb_pad32": table_pad32,
         "type_emb_padR": table_padR,
         "ents_t": ents_t[c], "idx_flat": idx_flat[c]}
        for c in range(NCORES)
    ]


def unshard_output(results):
    """Per-core result dicts -> full [B, N, EMB] f32 output (exact bf16 widen)."""
    outs = []
    for r in results:
        o = np.asarray(r["out"])
        if o.dtype != np.float32:
            o = _bf16_to_f32(o.view(np.uint16))
        outs.append(o.reshape(BP, N, EMB))
    return np.concatenate(outs, axis=0)


def _spot_check(out, entities, type_emb, n=256):
    """Cheap host-side sanity check on a random sample of entities: the
    embedding slots must match the bf16-rounded table row, slots 0/3 the
    bf16-rounded input channels."""
    rng = np.random.default_rng(1234)
    ef = entities.reshape(-1, 5)
    of = out.reshape(-1, EMB)
    sel = rng.integers(0, ef.shape[0], n)
    ids = ef[sel, 4].astype(np.int64)
    if not np.array_equal(of[sel, 4:], _bf16_to_f32(_f32_to_bf16(type_emb[ids]))):
        return False
    if not np.array_equal(of[sel, 0], _bf16_to_f32(_f32_to_bf16(ef[sel, 1]))):
        return False
    return bool(np.isfinite(of[sel]).all())


def kernel(entities, type_emb):
    from concourse.bass_utils import run_bass_kernel_spmd

    entities = np.ascontiguousarray(entities, dtype=np.float32)
    type_emb = np.ascontiguousarray(type_emb, dtype=np.float32)
    nc = build_nc()
    in_maps = shard_inputs(entities, type_emb)
    last_err = None
    for _attempt in range(3):
        try:
            res = run_bass_kernel_spmd(nc, in_maps, list(range(NCORES)))
            out = unshard_output(res.results)
            if _spot_check(out, entities, type_emb):
                return out
            last_err = RuntimeError("kernel output failed spot check")
        except Exception as e:  # flaky device sessions: retry
            last_err = e
    raise last_err
